# revision 25
# baseline (speedup 1.0000x reference)
"""MinLSTM layer on 8 Trainium2 NeuronCores.

Math (equivalent to the log-space reference, done in linear space):
    f_pre = x @ W_f.T + b_f ; i_pre = x @ W_i.T + b_i ; h_pre = x @ W_h.T + b_h
    sf = sigmoid(f_pre) ; si = sigmoid(i_pre)
    f = sf / (sf + si)                       # normalized forget gate
    i = 1 - f                                # = si / (sf + si)
    g = max(sigmoid(h_pre), h_pre + 0.5)     # == exp(log_g), exactly
    h_t = f_t * h_{t-1} + i_t * g_t,  h_0 = 1
The gates satisfy f in (0,1), g > 0, so h stays in a tame range and the
recurrence is numerically stable in fp32 (max rel err vs the fp32 log-space
reference ~6e-4 = the reference's own fp32 noise floor).

Sharding: 8 cores = batch(4) x hidden-halves(2). Core c handles batch b=c//2,
hidden slice [(c%2)*512, (c%2+1)*512). No cross-core communication; the scan
runs along T inside each core via the DVE TensorTensorScan instruction
(state = f*state - mv per step, mv = (f-1)*g = -i*g).

Device layout: gates computed as [h_part, t_free] via out = W_sliceT.T @ xT;
host pre-transposes x and W (numpy) and re-transposes the [512, 4096] per-core
output back to [T, Dh]. Matmuls run in 512-wide t-chunks (one PSUM bank);
elementwise+scan run in up-to-1024-wide super-chunks to amortize DVE fixed
overhead, with the scan carry passed as the previous chunk's last column.

Scheduling notes:
- x and W live in per-k tiles (contraction slices) so the PE's dependency on
  each matmul is one 256KB DMA, not a whole 2MB tensor: at startup the PE
  chases the HBM stream (~390 GB/s) instead of idling for all weights.
- The first super-chunk is gate-major (f for all h-tiles, then i, then h),
  k-outer, in DMA priority order (x0[k], W_f[k]) -> x1 -> W_i -> W_h; later
  chunks are h-tile-major with per-(gate, half) PSUM accumulation.
- 24 small warmup matmuls on a zeroed scratch tile fill the engine-preamble ->
  first-data window so the PE's HAM clock gate is at 2.4 GHz (not the 1.2 GHz
  cold rate) when real matmuls start.
- The final chunk's elementwise/scan runs at 256-wide grain to shorten the
  serial DVE chain after the last matmul.
"""

import sys

for _p in ("/opt/trn_rl_repo",):
    if _p not in sys.path:
        sys.path.append(_p)

import numpy as np

import concourse.bass as bass
import concourse.tile as tile
from concourse import bacc, mybir
from concourse.bass_utils import run_bass_kernel_spmd

B, T, DIN, DH = 4, 4096, 1024, 1024
N_CORES = 8
HSH = DH // 2          # 512 hidden channels per core
P = 128                # partitions
KT = DIN // P          # 8 contraction tiles
NT = 512               # matmul t-chunk (free dim, one PSUM bank)
IT = HSH // P          # 4 h-tiles per core
# elementwise/scan super-chunks (start, length); tail chunks smaller to
# shrink the end-of-kernel drain
CHUNKS = [(0, 1024), (1024, 1024), (2048, 1024), (3072, 512), (3584, 512)]

# float32r streams fp32 operands through the PE at bf16 rate when the moving
# free dim >= 256. Measured (K=128): mean rel err ~1e-3 vs fp64, ~16x better
# than bf16. Fallbacks: mybir.dt.float32 (4x slower, exact) / bfloat16.
MM_DT = mybir.dt.float32r

_COMPILED = None


def _build():
    AF = mybir.ActivationFunctionType
    OP = mybir.AluOpType
    f32 = mybir.dt.float32

    nc = bacc.Bacc("TRN2", target_bir_lowering=False, debug=False)

    xT = nc.dram_tensor("xT", [DIN, T], MM_DT, kind="ExternalInput").ap()
    wd = {g: nc.dram_tensor(f"w{g}", [DIN, HSH], MM_DT, kind="ExternalInput").ap()
          for g in ("f", "i", "h")}
    # packed per-partition scalars: [b_f | b_i | b_h | b_h+0.5], each (128, IT)
    biases = nc.dram_tensor("biases", [P, 4 * IT], f32, kind="ExternalInput").ap()
    out = nc.dram_tensor("out", [HSH, T], f32, kind="ExternalOutput").ap()

    # DRAM views: (KT*P, n) -> [p, k, n]
    xT_v = xT.rearrange("(k p) t -> p k t", p=P)
    w_v = {g: w.rearrange("(k p) h -> p k h", p=P) for g, w in wd.items()}

    with tile.TileContext(nc) as tc:
        with (
            tc.tile_pool(name="wpool", bufs=1) as wpool,
            tc.tile_pool(name="bpool", bufs=1) as bpool,
            tc.tile_pool(name="xpool", bufs=32) as xpool,
            tc.tile_pool(name="psum", bufs=8, space="PSUM") as pspool,
            tc.tile_pool(name="work", bufs=4) as work,
            tc.tile_pool(name="hpool", bufs=6) as hpool,
        ):
            bias_t = bpool.tile([P, 4 * IT], f32, tag="bias")

            # per-k weight tiles, resident all kernel
            wt = {g: [wpool.tile([P, HSH], MM_DT, tag=f"w{g}{k}", name=f"w{g}{k}_t")
                      for k in range(KT)] for g in ("f", "i", "h")}

            def dma_w(g):
                for k in range(KT):
                    nc.sync.dma_start(out=wt[g][k][:], in_=w_v[g][:, k, :])

            def x_ktiles(t0):
                """One [P, NT] tile per contraction slice k of a t-chunk."""
                xs = []
                for k in range(KT):
                    xk = xpool.tile([P, NT], MM_DT, tag="xk", name="xk_t")
                    nc.sync.dma_start(out=xk[:], in_=xT_v[:, k, t0:t0 + NT])
                    xs.append(xk)
                return xs

            def bias_ap(kind, i):
                return bias_t[:, kind * IT + i:kind * IT + i + 1]

            def chain(i, sf, si, sg, gt, J, t0, ne, grain=None):
                """Normalize gates, build -i*g, scan, and store chunk.

                grain < ne splits the elementwise+scan into sub-chunks so the
                last chunk's serial DVE chain off the critical tail is short.
                """
                grain = grain or ne
                for c0 in range(0, ne, grain):
                    cs = slice(c0, c0 + grain)
                    nc.vector.tensor_add(si[:, cs], sf[:, cs], si[:, cs])
                    r = work.tile([P, grain], f32, tag="sg", name="r_t")
                    nc.vector.reciprocal_approx_fast(out=r[:], in_=si[:, cs])
                    nc.vector.tensor_mul(sf[:, cs], sf[:, cs], r[:])      # f
                    nc.vector.scalar_tensor_tensor(                # mv=(f-1)*g
                        gt[:, cs], sf[:, cs], 1.0, gt[:, cs],
                        op0=OP.subtract, op1=OP.mult)
                    hc = hpool.tile([P, grain], f32, tag="h", name=f"h{i}_t")
                    init = 1.0 if J == 0 and c0 == 0 else hprev[i][:, -1:]
                    nc.vector.tensor_tensor_scan(
                        hc[:], sf[:, cs], gt[:, cs], init,
                        op0=OP.mult, op1=OP.subtract)
                    hprev[i] = hc
                    nc.sync.dma_start(
                        out=out[i * P:(i + 1) * P, t0 + c0:t0 + c0 + grain],
                        in_=hc[:])

            hprev = [None] * IT
            hsls = [slice(i * P, (i + 1) * P) for i in range(IT)]

            # Fill the preamble->first-data window (~6.5-11us) with small
            # warmup matmuls on a zeroed scratch tile so the PE's HAM clock
            # gate reaches 2.4 GHz before real matmuls start; J0's early
            # blocks then run warm instead of at the 1.2 GHz cold rate.
            scratch = bpool.tile([P, P], MM_DT, tag="scratch")
            nc.vector.memset(scratch[:].bitcast(mybir.dt.uint32), 0)
            pswarm = pspool.tile([P, P], f32, tag="ps", name="pswarm_t")
            for _ in range(18):
                nc.tensor.matmul(pswarm[:], lhsT=scratch[:], rhs=scratch[:],
                                 start=True, stop=True)

            # ---- J0: gate-major, k-outer; PE chases the input DMA stream ----
            t0, ne = CHUNKS[0]
            nhalf = ne // NT
            # priority order: (x_h0[k], x_h1[k], W_f[k]) trios, W_i, W_h.
            # Interleaving both halves per k doubles the matmul work enabled
            # per delivered byte during the DMA-bound f-phase (~94% PE duty).
            xcs = [[xpool.tile([P, NT], MM_DT, tag="xk", name="xk_t")
                    for _ in range(KT)] for _ in range(nhalf)]
            for k in range(KT):
                for h in range(nhalf):
                    th = t0 + h * NT
                    nc.sync.dma_start(out=xcs[h][k][:],
                                      in_=xT_v[:, k, th:th + NT])
                nc.sync.dma_start(out=wt["f"][k][:], in_=w_v["f"][:, k, :])
                if k == 0:
                    # bias is tiny and first needed by the ACTs at ~14us;
                    # issue it after the first matmul's dependencies
                    nc.sync.dma_start(out=bias_t[:], in_=biases[:])
            dma_w("i")
            dma_w("h")

            sf = [work.tile([P, ne], f32, tag="sf", name="sf_t") for _ in range(IT)]
            si = [work.tile([P, ne], f32, tag="si", name="si_t") for _ in range(IT)]
            sg = [work.tile([P, ne], f32, tag="sg", name="sg_t") for _ in range(IT)]
            gt = [work.tile([P, ne], f32, tag="gt", name="gt_t") for _ in range(IT)]
            # f-gate: both halves interleaved per k (8 live PSUM banks) so
            # the PE consumes the trio DMA stream at full rate
            psts2 = [[pspool.tile([P, NT], f32, tag="ps", name="ps_t")
                      for _ in range(IT)] for _ in range(nhalf)]
            for k in range(KT):
                for half in range(nhalf):
                    for pst, hsl in zip(psts2[half], hsls):
                        nc.tensor.matmul(
                            pst[:], lhsT=wt["f"][k][:, hsl],
                            rhs=xcs[half][k][:],
                            start=(k == 0), stop=(k == KT - 1))
            for half in range(nhalf):
                esl = slice(half * NT, (half + 1) * NT)
                for i in range(IT):
                    nc.scalar.activation(sf[i][:, esl], psts2[half][i][:],
                                         AF.Sigmoid, bias=bias_ap(0, i), scale=1.0)
            for gate, dsts, bk in (("i", si, 1), ("h", sg, 2)):
                for half in range(nhalf):
                    esl = slice(half * NT, (half + 1) * NT)
                    psts = [pspool.tile([P, NT], f32, tag="ps", name="ps_t")
                            for _ in range(IT)]
                    for k in range(KT):
                        for pst, hsl in zip(psts, hsls):
                            nc.tensor.matmul(
                                pst[:], lhsT=wt[gate][k][:, hsl],
                                rhs=xcs[half][k][:],
                                start=(k == 0), stop=(k == KT - 1))
                    for i in range(IT):
                        nc.scalar.activation(dsts[i][:, esl], psts[i][:], AF.Sigmoid,
                                             bias=bias_ap(bk, i), scale=1.0)
                        if gate == "h":
                            nc.vector.scalar_tensor_tensor(
                                gt[i][:, esl], psts[i][:], bias_ap(3, i),
                                sg[i][:, esl], op0=OP.add, op1=OP.max)
            for i in range(IT):
                chain(i, sf[i], si[i], sg[i], gt[i], 0, t0, ne)

            # ---- J1+: h-tile-major units ----
            for J, (t0, ne) in enumerate(CHUNKS[1:], start=1):
                nhalf = ne // NT
                xcs = [x_ktiles(t0 + h * NT) for h in range(nhalf)]
                for i in range(IT):
                    hsl = hsls[i]
                    sf = work.tile([P, ne], f32, tag="sf", name="sf_t")
                    si = work.tile([P, ne], f32, tag="si", name="si_t")
                    sg = work.tile([P, ne], f32, tag="sg", name="sg_t")
                    gt = work.tile([P, ne], f32, tag="gt", name="gt_t")
                    for half in range(nhalf):
                        esl = slice(half * NT, (half + 1) * NT)
                        for gate, dst, bk in (("f", sf, 0), ("i", si, 1),
                                              ("h", sg, 2)):
                            pst = pspool.tile([P, NT], f32, tag="ps", name="ps_t")
                            for k in range(KT):
                                nc.tensor.matmul(
                                    pst[:], lhsT=wt[gate][k][:, hsl],
                                    rhs=xcs[half][k][:],
                                    start=(k == 0), stop=(k == KT - 1))
                            nc.scalar.activation(dst[:, esl], pst[:],
                                                 AF.Sigmoid, bias=bias_ap(bk, i),
                                                 scale=1.0)
                            if gate == "h":
                                nc.vector.scalar_tensor_tensor(
                                    gt[:, esl], pst[:], bias_ap(3, i),
                                    sg[:, esl], op0=OP.add, op1=OP.max)
                    chain(i, sf, si, sg, gt, J, t0, ne,
                          grain=256 if J == len(CHUNKS) - 1 else None)

    nc.compile()
    return nc


def _in_maps(x, W_f, b_f, W_i, b_i, W_h, b_h):
    x = np.asarray(x, np.float32)
    wT = {g: np.ascontiguousarray(np.asarray(w, np.float32).T)
          for g, w in (("f", W_f), ("i", W_i), ("h", W_h))}
    bs = {g: np.asarray(b, np.float32) for g, b in (("f", b_f), ("i", b_i), ("h", b_h))}

    maps = []
    for c in range(N_CORES):
        b, hh = divmod(c, 2)
        hsl = slice(hh * HSH, (hh + 1) * HSH)
        bias_pack = np.concatenate([
            bs["f"][hsl].reshape(IT, P).T,
            bs["i"][hsl].reshape(IT, P).T,
            bs["h"][hsl].reshape(IT, P).T,
            (bs["h"][hsl] + 0.5).reshape(IT, P).T,
        ], axis=1)
        maps.append({
            "xT": np.ascontiguousarray(x[b].T),
            "wf": np.ascontiguousarray(wT["f"][:, hsl]),
            "wi": np.ascontiguousarray(wT["i"][:, hsl]),
            "wh": np.ascontiguousarray(wT["h"][:, hsl]),
            "biases": np.ascontiguousarray(bias_pack, dtype=np.float32),
        })
    return maps


def kernel(x, W_f, b_f, W_i, b_i, W_h, b_h):
    global _COMPILED
    if _COMPILED is None:
        _COMPILED = _build()
    nc = _COMPILED

    res = run_bass_kernel_spmd(
        nc, _in_maps(x, W_f, b_f, W_i, b_i, W_h, b_h), list(range(N_CORES)))

    full = np.empty((B, T, DH), np.float32)
    for c in range(N_CORES):
        b, hh = divmod(c, 2)
        full[b, :, hh * HSH:(hh + 1) * HSH] = res.results[c]["out"].T
    return full


# revision 26
# speedup vs baseline: 1.0158x; 1.0158x over previous
"""MinLSTM layer on 8 Trainium2 NeuronCores.

Math (equivalent to the log-space reference, done in linear space):
    f_pre = x @ W_f.T + b_f ; i_pre = x @ W_i.T + b_i ; h_pre = x @ W_h.T + b_h
    sf = sigmoid(f_pre) ; si = sigmoid(i_pre)
    f = sf / (sf + si)                       # normalized forget gate
    i = 1 - f                                # = si / (sf + si)
    g = max(sigmoid(h_pre), h_pre + 0.5)     # == exp(log_g), exactly
    h_t = f_t * h_{t-1} + i_t * g_t,  h_0 = 1
The gates satisfy f in (0,1), g > 0, so h stays in a tame range and the
recurrence is numerically stable in fp32 (max rel err vs the fp32 log-space
reference ~6e-4 = the reference's own fp32 noise floor).

Sharding: 8 cores = batch(4) x hidden-halves(2). Core c handles batch b=c//2,
hidden slice [(c%2)*512, (c%2+1)*512). No cross-core communication; the scan
runs along T inside each core via the DVE TensorTensorScan instruction
(state = f*state - mv per step, mv = (f-1)*g = -i*g).

Device layout: gates computed as [h_part, t_free] via out = W_sliceT.T @ xT;
host pre-transposes x and W (numpy) and re-transposes the [512, 4096] per-core
output back to [T, Dh]. Matmuls run in 512-wide t-chunks (one PSUM bank);
elementwise+scan run in up-to-1024-wide super-chunks to amortize DVE fixed
overhead, with the scan carry passed as the previous chunk's last column.

Scheduling notes:
- x and W live in per-k tiles (contraction slices) so the PE's dependency on
  each matmul is one 256KB DMA, not a whole 2MB tensor: at startup the PE
  chases the HBM stream (~390 GB/s) instead of idling for all weights.
- The first super-chunk is gate-major (f for all h-tiles, then i, then h),
  k-outer, in DMA priority order (x0[k], W_f[k]) -> x1 -> W_i -> W_h; later
  chunks are h-tile-major with per-(gate, half) PSUM accumulation.
- 24 small warmup matmuls on a zeroed scratch tile fill the engine-preamble ->
  first-data window so the PE's HAM clock gate is at 2.4 GHz (not the 1.2 GHz
  cold rate) when real matmuls start.
- The final chunk's elementwise/scan runs at 256-wide grain to shorten the
  serial DVE chain after the last matmul.
"""

import sys

for _p in ("/opt/trn_rl_repo",):
    if _p not in sys.path:
        sys.path.append(_p)

import numpy as np

import concourse.bass as bass
import concourse.tile as tile
from concourse import bacc, mybir
from concourse.bass_utils import run_bass_kernel_spmd

B, T, DIN, DH = 4, 4096, 1024, 1024
N_CORES = 8
HSH = DH // 2          # 512 hidden channels per core
P = 128                # partitions
KT = DIN // P          # 8 contraction tiles
NT = 512               # matmul t-chunk (free dim, one PSUM bank)
IT = HSH // P          # 4 h-tiles per core
# elementwise/scan super-chunks (start, length); tail chunks smaller to
# shrink the end-of-kernel drain
CHUNKS = [(0, 1024), (1024, 1024), (2048, 1024), (3072, 512), (3584, 512)]

# float32r streams fp32 operands through the PE at bf16 rate when the moving
# free dim >= 256. Measured (K=128): mean rel err ~1e-3 vs fp64, ~16x better
# than bf16. Fallbacks: mybir.dt.float32 (4x slower, exact) / bfloat16.
MM_DT = mybir.dt.float32r

_COMPILED = None


def _build():
    AF = mybir.ActivationFunctionType
    OP = mybir.AluOpType
    f32 = mybir.dt.float32

    nc = bacc.Bacc("TRN2", target_bir_lowering=False, debug=False)

    xT = nc.dram_tensor("xT", [DIN, T], MM_DT, kind="ExternalInput").ap()
    wd = {g: nc.dram_tensor(f"w{g}", [DIN, HSH], MM_DT, kind="ExternalInput").ap()
          for g in ("f", "i", "h")}
    # packed per-partition scalars: [b_f | b_i | b_h | b_h+0.5], each (128, IT)
    biases = nc.dram_tensor("biases", [P, 4 * IT], f32, kind="ExternalInput").ap()
    out = nc.dram_tensor("out", [HSH, T], f32, kind="ExternalOutput").ap()

    # DRAM views: (KT*P, n) -> [p, k, n]
    xT_v = xT.rearrange("(k p) t -> p k t", p=P)
    w_v = {g: w.rearrange("(k p) h -> p k h", p=P) for g, w in wd.items()}

    with tile.TileContext(nc) as tc:
        with (
            tc.tile_pool(name="wpool", bufs=1) as wpool,
            tc.tile_pool(name="bpool", bufs=1) as bpool,
            tc.tile_pool(name="xpool", bufs=32) as xpool,
            tc.tile_pool(name="psum", bufs=8, space="PSUM") as pspool,
            tc.tile_pool(name="work", bufs=4) as work,
            tc.tile_pool(name="hpool", bufs=6) as hpool,
        ):
            bias_t = bpool.tile([P, 4 * IT], f32, tag="bias")

            # per-k weight tiles, resident all kernel
            wt = {g: [wpool.tile([P, HSH], MM_DT, tag=f"w{g}{k}", name=f"w{g}{k}_t")
                      for k in range(KT)] for g in ("f", "i", "h")}

            def dma_w(g):
                for k in range(KT):
                    nc.sync.dma_start(out=wt[g][k][:], in_=w_v[g][:, k, :])

            def x_ktiles(t0):
                """One [P, NT] tile per contraction slice k of a t-chunk."""
                xs = []
                for k in range(KT):
                    xk = xpool.tile([P, NT], MM_DT, tag="xk", name="xk_t")
                    nc.sync.dma_start(out=xk[:], in_=xT_v[:, k, t0:t0 + NT])
                    xs.append(xk)
                return xs

            def bias_ap(kind, i):
                return bias_t[:, kind * IT + i:kind * IT + i + 1]

            def chain(i, sf, si, sg, gt, J, t0, ne, grain=None):
                """Normalize gates, build -i*g, scan, and store chunk.

                grain < ne splits the elementwise+scan into sub-chunks so the
                last chunk's serial DVE chain off the critical tail is short.
                """
                grain = grain or ne
                for c0 in range(0, ne, grain):
                    cs = slice(c0, c0 + grain)
                    nc.vector.tensor_add(si[:, cs], sf[:, cs], si[:, cs])
                    r = work.tile([P, grain], f32, tag="sg", name="r_t")
                    nc.vector.reciprocal_approx_fast(out=r[:], in_=si[:, cs])
                    nc.vector.tensor_mul(sf[:, cs], sf[:, cs], r[:])      # f
                    nc.vector.scalar_tensor_tensor(                # mv=(f-1)*g
                        gt[:, cs], sf[:, cs], 1.0, gt[:, cs],
                        op0=OP.subtract, op1=OP.mult)
                    hc = hpool.tile([P, grain], f32, tag="h", name=f"h{i}_t")
                    init = 1.0 if J == 0 and c0 == 0 else hprev[i][:, -1:]
                    nc.vector.tensor_tensor_scan(
                        hc[:], sf[:, cs], gt[:, cs], init,
                        op0=OP.mult, op1=OP.subtract)
                    hprev[i] = hc
                    nc.sync.dma_start(
                        out=out[i * P:(i + 1) * P, t0 + c0:t0 + c0 + grain],
                        in_=hc[:])

            hprev = [None] * IT
            hsls = [slice(i * P, (i + 1) * P) for i in range(IT)]

            # Fill the preamble->first-data window (~6.5-11us) with small
            # warmup matmuls on a zeroed scratch tile so the PE's HAM clock
            # gate reaches 2.4 GHz before real matmuls start; J0's early
            # blocks then run warm instead of at the 1.2 GHz cold rate.
            scratch = bpool.tile([P, P], MM_DT, tag="scratch")
            nc.vector.memset(scratch[:].bitcast(mybir.dt.uint32), 0)
            pswarm = pspool.tile([P, P], f32, tag="ps", name="pswarm_t")
            for _ in range(18):
                nc.tensor.matmul(pswarm[:], lhsT=scratch[:], rhs=scratch[:],
                                 start=True, stop=True)

            # ---- J0: gate-major, k-outer; PE chases the input DMA stream ----
            t0, ne = CHUNKS[0]
            nhalf = ne // NT
            # priority order: (x_h0[k], W_f[k]) pairs, x_h1, W_i, W_h
            xcs = [[xpool.tile([P, NT], MM_DT, tag="xk", name="xk_t")
                    for _ in range(KT)] for _ in range(nhalf)]
            for k in range(KT):
                nc.sync.dma_start(out=xcs[0][k][:], in_=xT_v[:, k, t0:t0 + NT])
                nc.sync.dma_start(out=wt["f"][k][:], in_=w_v["f"][:, k, :])
                if k == 0:
                    # bias is tiny and first needed by the ACTs at ~14us;
                    # issue it after the first matmul's dependencies
                    nc.sync.dma_start(out=bias_t[:], in_=biases[:])
            for h in range(1, nhalf):
                th = t0 + h * NT
                for k in range(KT):
                    nc.sync.dma_start(out=xcs[h][k][:], in_=xT_v[:, k, th:th + NT])
            dma_w("i")
            dma_w("h")

            sf = [work.tile([P, ne], f32, tag="sf", name="sf_t") for _ in range(IT)]
            si = [work.tile([P, ne], f32, tag="si", name="si_t") for _ in range(IT)]
            sg = [work.tile([P, ne], f32, tag="sg", name="sg_t") for _ in range(IT)]
            gt = [work.tile([P, ne], f32, tag="gt", name="gt_t") for _ in range(IT)]
            for gate, dsts, bk in (("f", sf, 0), ("i", si, 1), ("h", sg, 2)):
                for half in range(nhalf):
                    esl = slice(half * NT, (half + 1) * NT)
                    psts = [pspool.tile([P, NT], f32, tag="ps", name="ps_t")
                            for _ in range(IT)]
                    for k in range(KT):
                        for pst, hsl in zip(psts, hsls):
                            nc.tensor.matmul(
                                pst[:], lhsT=wt[gate][k][:, hsl],
                                rhs=xcs[half][k][:],
                                start=(k == 0), stop=(k == KT - 1))
                    for i in range(IT):
                        nc.scalar.activation(dsts[i][:, esl], psts[i][:], AF.Sigmoid,
                                             bias=bias_ap(bk, i), scale=1.0)
                        if gate == "h":
                            nc.vector.scalar_tensor_tensor(
                                gt[i][:, esl], psts[i][:], bias_ap(3, i),
                                sg[i][:, esl], op0=OP.add, op1=OP.max)
            for i in range(IT):
                chain(i, sf[i], si[i], sg[i], gt[i], 0, t0, ne)

            # ---- J1+: h-tile-major units ----
            for J, (t0, ne) in enumerate(CHUNKS[1:], start=1):
                nhalf = ne // NT
                xcs = [x_ktiles(t0 + h * NT) for h in range(nhalf)]
                for i in range(IT):
                    hsl = hsls[i]
                    sf = work.tile([P, ne], f32, tag="sf", name="sf_t")
                    si = work.tile([P, ne], f32, tag="si", name="si_t")
                    sg = work.tile([P, ne], f32, tag="sg", name="sg_t")
                    gt = work.tile([P, ne], f32, tag="gt", name="gt_t")
                    for half in range(nhalf):
                        esl = slice(half * NT, (half + 1) * NT)
                        for gate, dst, bk in (("f", sf, 0), ("i", si, 1),
                                              ("h", sg, 2)):
                            pst = pspool.tile([P, NT], f32, tag="ps", name="ps_t")
                            for k in range(KT):
                                nc.tensor.matmul(
                                    pst[:], lhsT=wt[gate][k][:, hsl],
                                    rhs=xcs[half][k][:],
                                    start=(k == 0), stop=(k == KT - 1))
                            nc.scalar.activation(dst[:, esl], pst[:],
                                                 AF.Sigmoid, bias=bias_ap(bk, i),
                                                 scale=1.0)
                            if gate == "h":
                                nc.vector.scalar_tensor_tensor(
                                    gt[:, esl], pst[:], bias_ap(3, i),
                                    sg[:, esl], op0=OP.add, op1=OP.max)
                    chain(i, sf, si, sg, gt, J, t0, ne,
                          grain=256 if J == len(CHUNKS) - 1 else None)

    nc.compile()
    return nc


def _in_maps(x, W_f, b_f, W_i, b_i, W_h, b_h):
    x = np.asarray(x, np.float32)
    wT = {g: np.ascontiguousarray(np.asarray(w, np.float32).T)
          for g, w in (("f", W_f), ("i", W_i), ("h", W_h))}
    bs = {g: np.asarray(b, np.float32) for g, b in (("f", b_f), ("i", b_i), ("h", b_h))}

    maps = []
    for c in range(N_CORES):
        b, hh = divmod(c, 2)
        hsl = slice(hh * HSH, (hh + 1) * HSH)
        bias_pack = np.concatenate([
            bs["f"][hsl].reshape(IT, P).T,
            bs["i"][hsl].reshape(IT, P).T,
            bs["h"][hsl].reshape(IT, P).T,
            (bs["h"][hsl] + 0.5).reshape(IT, P).T,
        ], axis=1)
        maps.append({
            "xT": np.ascontiguousarray(x[b].T),
            "wf": np.ascontiguousarray(wT["f"][:, hsl]),
            "wi": np.ascontiguousarray(wT["i"][:, hsl]),
            "wh": np.ascontiguousarray(wT["h"][:, hsl]),
            "biases": np.ascontiguousarray(bias_pack, dtype=np.float32),
        })
    return maps


def kernel(x, W_f, b_f, W_i, b_i, W_h, b_h):
    global _COMPILED
    if _COMPILED is None:
        _COMPILED = _build()
    nc = _COMPILED

    res = run_bass_kernel_spmd(
        nc, _in_maps(x, W_f, b_f, W_i, b_i, W_h, b_h), list(range(N_CORES)))

    full = np.empty((B, T, DH), np.float32)
    for c in range(N_CORES):
        b, hh = divmod(c, 2)
        full[b, :, hh * HSH:(hh + 1) * HSH] = res.results[c]["out"].T
    return full
